# revision 13
# baseline (speedup 1.0000x reference)
"""Trainium2 Bass kernel for nn_FCPairedLayer (gnn_message_passing).

Reference computation:
    v[b,i,j] = concat(x_i, x_j, x_{i-1}*m1, x_{j+1}*m1, x_{i+1}*m2, x_{j-1}*m2)
    y[b,i,j] = W2 @ relu(W1 @ v + b1) + b2        (scalar output per pair)
with m1 = [i>=1][j<=N-2], m2 = [i<=N-2][j>=1].

W1 @ v splits into row-only and column-only terms; per batch define
    R[:,i] = W1_a x_i + W1_c x_{i-1} + W1_e x_{i+1} + b1     (shifts masked)
    C[:,j] = W1_b x_j + W1_d x_{j+1} + W1_f x_{j-1}
so that for interior cells  y[i,j] = W2 @ relu(R_i + C_j) + b2.
Boundary corrections:
    column j=0   uses R0   = W1_a x_i + W1_c x_{i-1} + b1     (drop e-term)
    column j=383 uses R383 = W1_a x_i + W1_e x_{i+1} + b1     (drop c-term)
    row i=0      uses CA   = W1_b x_j + W1_f x_{j-1}          (drop d-term)
    row i=383    uses CB   = W1_b x_j + W1_d x_{j+1}          (drop f-term)
    corners (0,0)/(383,383) need both (handled by patching R0/R383 columns).

Sharding: 8 cores, 48 output rows (i) each, both batches; every core gets the
full (transposed, zero-padded) x so the +-1 shifts are just AP column offsets.
The program is SPMD-uniform; core specialization (row-0/row-383 corrections)
enters only through per-core input data (S_CA3/S_CB3 zeroed on edge cores,
per-core x column slices).

Per (b,i) row the hidden tile relu(R_i + C) [128h x 384j] is produced by one
fused op (add + max 0) on DVE / ACT / GPSIMD (weighted round-robin), then
reduced against W2 by a PE matmul whose stationary is a [128,32] slice of a
zero slab with w2 at one column -> each row's result lands on its own PSUM
partition of a single [96, 384] accumulator bank (b2 pre-accumulated by a
contract-1 matmul).  One DVE copy extracts everything to SBUF.
"""

import ml_dtypes
import numpy as np
from contextlib import ExitStack

import concourse.bass as bass
import concourse.bacc as bacc
import concourse.tile as tile
from concourse import mybir
from concourse.bass_utils import run_bass_kernel_spmd

B, N, CIN, H = 2, 384, 64, 128
NCORES = 8
RPC = N // NCORES  # rows (i) per core = 48
ROWS = B * RPC     # (b, i) rows per core = 96

F32 = mybir.dt.float32
F32R = mybir.dt.float32r
BF16 = mybir.dt.bfloat16

# dtype of the C-side tensors and the relu'd hidden tiles.
CDT = BF16
MDT = BF16

ADD = mybir.AluOpType.add
MAX = mybir.AluOpType.max
RELU = mybir.ActivationFunctionType.Relu
IDENT = mybir.ActivationFunctionType.Identity

# main-loop row split across elementwise engines
N_DVE, N_ACT, N_POOL = 64, 15, 17
assert N_DVE + N_ACT + N_POOL == ROWS

USE_TILE_POSITION = True


def _engine_sequence():
    """Bresenham-interleave the 96 rows across the three engines."""
    quota = {"v": N_DVE, "s": N_ACT, "g": N_POOL}
    err = {e: 0.0 for e in quota}
    seq = []
    for _ in range(ROWS):
        for e in quota:
            err[e] += quota[e]
        best = max(err, key=lambda e: err[e])
        seq.append(best)
        err[best] -= ROWS
    return seq


def build_program(b2_value: float):
    """Build the SPMD Bass program (same NEFF for all 8 cores)."""
    nc = bacc.Bacc(
        "TRN2", target_bir_lowering=False, debug=False,
        enable_asserts=False, num_devices=NCORES,
    )
    # ---- DRAM I/O ----
    # constants are bundled by partition count so each lands in one DMA
    W64 = B * (N + 2) + B * (RPC + 2) + 8 * H   # xp | xsl | S_a..S_f,SCA3,SCB3
    W128 = 63 + 1                               # w2slab | w2t
    d_b64 = nc.dram_tensor("b64", [CIN, W64], F32R, kind="ExternalInput").ap()
    d_b128 = nc.dram_tensor("b128", [H, W128], BF16, kind="ExternalInput").ap()
    d_b1 = nc.dram_tensor("b1c", [H, 1], F32, kind="ExternalInput").ap()
    d_y = nc.dram_tensor("y", [B, RPC, N], F32, kind="ExternalOutput").ap()

    eng_seq = _engine_sequence()

    with tile.TileContext(nc) as tc, ExitStack() as ctx:
        consts = ctx.enter_context(tc.tile_pool(name="consts", bufs=1))
        cpool = ctx.enter_context(tc.tile_pool(name="cmats", bufs=1))
        rpool = ctx.enter_context(tc.tile_pool(name="rmats", bufs=1))
        mpool = ctx.enter_context(tc.tile_pool(name="mtiles", bufs=9))
        ypool = ctx.enter_context(tc.tile_pool(name="yout", bufs=1))
        ps = ctx.enter_context(tc.tile_pool(name="ps", bufs=6, space="PSUM"))
        yps_pool = ctx.enter_context(tc.tile_pool(name="yps", bufs=2, space="PSUM"))

        # ---- load constants ----
        W64 = B * (N + 2) + B * (RPC + 2) + 8 * H
        b64 = consts.tile([CIN, W64], F32R, tag="b64", name="b64")
        nc.sync.dma_start(b64[:, :], d_b64)
        b128 = consts.tile([H, 63 + 1], BF16, tag="b128", name="b128")
        nc.sync.dma_start(b128[:, :], d_b128)
        b1 = consts.tile([H, 1], F32, tag="b1c", name="b1c")
        nc.sync.dma_start(b1[:, 0:1], d_b1)

        off = 0
        xp, xsl = [], []
        for b in range(B):
            xp.append(b64[:, off:off + N + 2])
            off += N + 2
        for b in range(B):
            xsl.append(b64[:, off:off + RPC + 2])
            off += RPC + 2
        S = {}
        for k in "abcdef":
            S[k] = b64[:, off:off + H]
            off += H
        SCA3 = b64[:, off:off + H]
        off += H
        SCB3 = b64[:, off:off + H]
        off += H
        assert off == W64
        w2slab = b128[:, 0:63]
        w2 = b128[:, 63:64]

        def mmr(out_ap, lhsT_ap, rhs_ap, start, stop, tile_position=None):
            nc.tensor.matmul(out_ap, lhsT_ap, rhs_ap,
                             start=start, stop=stop, tile_position=tile_position)

        # ---- per-batch setup: C, CA, CB, R, R0, R383 ----
        C_sb, CA_sb, CB_sb = [], [], []
        R_sb, R0_sb, R383_sb = [], [], []
        c0_f32, c383_f32 = [], []  # fp32 C columns 0/383 for the column fixes
        for b in range(B):
            xpb, xslb = xp[b], xsl[b]
            # C = S_b·x + S_d·x(+1) + S_f·x(-1)   [128, 384] in PSUM
            C_ps = ps.tile([H, N], F32, tag="ps", name="ps")
            mmr(C_ps[:, :], S["b"], xpb[:, 1:N + 1], True, False)
            mmr(C_ps[:, :], S["d"], xpb[:, 2:N + 2], False, False)
            mmr(C_ps[:, :], S["f"], xpb[:, 0:N], False, True)
            # CA = S_b·x + S_f·x(-1) + S_CA3·x(+1)  (S_CA3 = S_d, or 0 on core 0)
            CA_ps = ps.tile([H, N], F32, tag="ps", name="ps")
            mmr(CA_ps[:, :], S["b"], xpb[:, 1:N + 1], True, False)
            mmr(CA_ps[:, :], S["f"], xpb[:, 0:N], False, False)
            mmr(CA_ps[:, :], SCA3, xpb[:, 2:N + 2], False, True)
            # CB = S_b·x + S_d·x(+1) + S_CB3·x(-1)  (S_CB3 = S_f, or 0 on core 7)
            CB_ps = ps.tile([H, N], F32, tag="ps", name="ps")
            mmr(CB_ps[:, :], S["b"], xpb[:, 1:N + 1], True, False)
            mmr(CB_ps[:, :], S["d"], xpb[:, 2:N + 2], False, False)
            mmr(CB_ps[:, :], SCB3, xpb[:, 0:N], False, True)

            # R-side, on this core's 48-row slice
            R_ps = ps.tile([H, RPC], F32, tag="ps", name="ps")
            mmr(R_ps[:, :], S["a"], xslb[:, 1:RPC + 1], True, False)
            mmr(R_ps[:, :], S["c"], xslb[:, 0:RPC], False, True)
            r0 = rpool.tile([H, RPC], F32, tag=f"R0_{b}", name=f"R0_{b}")
            nc.scalar.activation(r0[:, :], R_ps[:, :], IDENT, bias=b1[:, 0:1])
            # continue accumulating the e-term onto the same PSUM -> full R
            mmr(R_ps[:, :], S["e"], xslb[:, 2:RPC + 2], False, True)
            r = rpool.tile([H, RPC], F32, tag=f"R_{b}", name=f"R_{b}")
            nc.scalar.activation(r[:, :], R_ps[:, :], IDENT, bias=b1[:, 0:1])
            R383_ps = ps.tile([H, RPC], F32, tag="ps", name="ps")
            mmr(R383_ps[:, :], S["a"], xslb[:, 1:RPC + 1], True, False)
            mmr(R383_ps[:, :], S["e"], xslb[:, 2:RPC + 2], False, True)
            r383 = rpool.tile([H, RPC], F32, tag=f"R383_{b}", name=f"R383_{b}")
            nc.scalar.activation(r383[:, :], R383_ps[:, :], IDENT, bias=b1[:, 0:1])
            R_sb.append(r)
            R0_sb.append(r0)
            R383_sb.append(r383)

            # copies PSUM -> SBUF (convert to CDT)
            c = cpool.tile([H, N], CDT, tag=f"C_{b}", name=f"C_{b}")
            nc.vector.tensor_copy(c[:, :], C_ps[:, :])
            ca = cpool.tile([H, N], CDT, tag=f"CA_{b}", name=f"CA_{b}")
            nc.vector.tensor_copy(ca[:, :], CA_ps[:, :])
            cb = cpool.tile([H, N], CDT, tag=f"CB_{b}", name=f"CB_{b}")
            nc.vector.tensor_copy(cb[:, :], CB_ps[:, :])
            C_sb.append(c)
            CA_sb.append(ca)
            CB_sb.append(cb)
            cc0 = rpool.tile([H, 1], F32, tag=f"c0_{b}", name=f"c0_{b}")
            nc.vector.tensor_copy(cc0[:, :], C_ps[:, 0:1])
            cc383 = rpool.tile([H, 1], F32, tag=f"c383_{b}", name=f"c383_{b}")
            nc.vector.tensor_copy(cc383[:, :], C_ps[:, N - 1:N])
            c0_f32.append(cc0)
            c383_f32.append(cc383)

            # corner patches: R0[:,0] += CA[:,0]-C[:,0]; R383[:,-1] += CB[:,-1]-C[:,-1]
            dca = rpool.tile([H, 1], F32, tag=f"dca_{b}", name=f"dca_{b}")
            nc.vector.tensor_sub(dca[:, :], ca[:, 0:1], c[:, 0:1])
            nc.vector.tensor_add(r0[:, 0:1], r0[:, 0:1], dca[:, :])
            dcb = rpool.tile([H, 1], F32, tag=f"dcb_{b}", name=f"dcb_{b}")
            nc.vector.tensor_sub(dcb[:, :], cb[:, N - 1:N], c[:, N - 1:N])
            nc.vector.tensor_add(r383[:, RPC - 1:RPC], r383[:, RPC - 1:RPC],
                                 dcb[:, :])

        # ---- main loop ----
        # All 96 row-results stack on distinct partitions of one PSUM bank.
        yacc = yps_pool.tile([ROWS, N], F32, tag="yacc", name="yacc")

        # visit rows rotating across the three 32-partition groups so the PE
        # array overlaps streams (different col_grp => different sub-arrays)
        visit = [(k % 3) * 32 + k // 3 for k in range(ROWS)]
        for k in range(ROWS):
            p = visit[k]
            b, i = divmod(p, RPC)
            if i == 0:
                cin = CA_sb[b]
            elif i == RPC - 1:
                cin = CB_sb[b]
            else:
                cin = C_sb[b]
            eng = eng_seq[k]
            m = mpool.tile([H, N], MDT, tag="m", name="m")
            rcol = R_sb[b][:, i:i + 1]
            if eng == "v":
                nc.vector.tensor_scalar(m[:, :], cin[:, :], rcol, 0.0, ADD, MAX)
            elif eng == "g":
                nc.gpsimd.tensor_scalar(m[:, :], cin[:, :], rcol, 0.0, ADD, MAX)
            else:
                nc.scalar.activation(m[:, :], cin[:, :], RELU, bias=rcol)
            g, c = divmod(p, 32)
            # stationary [128, 32] with w2 at column c: sliding slab slice
            stat = w2slab[:, 31 - c: 63 - c]
            tp = (0, 32 * g) if USE_TILE_POSITION else None
            nc.tensor.matmul(yacc[32 * g:32 * g + 32, :], stat, m[:, :],
                             start=k < 3, stop=k >= ROWS - 3, tile_position=tp)

        # extract all 96 rows at once, folding in +b2
        Y = ypool.tile([ROWS, N], F32, tag="Y", name="Y")
        nc.vector.tensor_scalar_add(Y[:, :], yacc[:, :], float(b2_value))

        # ---- boundary columns j=0 and j=383 (both batches in one matmul) ----
        for col in (0, N - 1):
            mc = mpool.tile([H, ROWS], MDT, tag="mcol", name="mcol")
            for b in range(B):
                rt = R0_sb[b] if col == 0 else R383_sb[b]
                csc = c0_f32[b] if col == 0 else c383_f32[b]
                nc.vector.tensor_scalar(mc[:, b * RPC:(b + 1) * RPC], rt[:, :],
                                        csc[:, :], 0.0, ADD, MAX)
            yc_ps = ps.tile([ROWS, 1], F32, tag="ps", name="ps")
            nc.tensor.matmul(yc_ps[:, :], mc[:, :], w2, start=True, stop=True)
            nc.vector.tensor_scalar_add(Y[:, col:col + 1], yc_ps[:, :],
                                        float(b2_value))

        # ---- store ----
        nc.sync.dma_start(d_y.flatten_outer_dims(), Y[:, :])

    nc.compile()
    return nc


def _prep_inputs(x, W1, b1, W2, b2):
    """Host-side restructuring (layout only, no FLOPs)."""
    x = np.asarray(x, np.float32)
    W1 = np.asarray(W1, np.float32)
    b1 = np.asarray(b1, np.float32)
    W2 = np.asarray(W2, np.float32)
    b2v = float(np.asarray(b2).reshape(-1)[0])
    xp = np.zeros((B, CIN, N + 2), np.float32)
    xp[:, :, 1:N + 1] = x.transpose(0, 2, 1)
    S = {k: np.ascontiguousarray(W1[:, 64 * i:64 * (i + 1)].T)
         for i, k in enumerate("abcdef")}
    w2slab = np.zeros((H, 63), np.float32)
    w2slab[:, 31] = W2.reshape(H)
    b128 = np.concatenate([w2slab, W2.reshape(1, H).T], axis=1)
    b128 = b128.astype(ml_dtypes.bfloat16)
    b1c = np.ascontiguousarray(b1.reshape(H, 1))
    return xp, S, b128, b1c, b2v


def kernel(x, W1, b1, W2, b2, trace=False):
    xp, S, b128, b1c, b2v = _prep_inputs(x, W1, b1, W2, b2)
    nc = build_program(b2v)

    zeros_s = np.zeros((CIN, H), np.float32)
    in_maps = []
    for c in range(NCORES):
        lo = c * RPC
        xsl = [xp[b, :, lo:lo + RPC + 2] for b in range(B)]
        sca3 = zeros_s if c == 0 else S["d"]
        scb3 = zeros_s if c == NCORES - 1 else S["f"]
        b64 = np.concatenate(
            [xp[0], xp[1]] + xsl + [S[k] for k in "abcdef"] + [sca3, scb3],
            axis=1)
        in_maps.append({
            "b64": np.ascontiguousarray(b64),
            "b128": b128,
            "b1c": b1c,
        })

    res = run_bass_kernel_spmd(nc, in_maps, core_ids=list(range(NCORES)),
                               trace=trace)
    y = np.concatenate([res.results[c]["y"] for c in range(NCORES)], axis=1)
    y = y.reshape(B, N, N, 1).astype(np.float32)
    if trace:
        return y, res
    return y


# revision 16
# speedup vs baseline: 3.1326x; 3.1326x over previous
"""Trainium2 Bass kernel for nn_FCPairedLayer (gnn_message_passing).

Reference computation:
    v[b,i,j] = concat(x_i, x_j, x_{i-1}*m1, x_{j+1}*m1, x_{i+1}*m2, x_{j-1}*m2)
    y[b,i,j] = W2 @ relu(W1 @ v + b1) + b2        (scalar output per pair)
with m1 = [i>=1][j<=N-2], m2 = [i<=N-2][j>=1].

W1 @ v splits into row-only and column-only terms; per batch define
    R[:,i] = W1_a x_i + W1_c x_{i-1} + W1_e x_{i+1} + b1     (shifts masked)
    C[:,j] = W1_b x_j + W1_d x_{j+1} + W1_f x_{j-1}
so that for interior cells  y[i,j] = W2 @ relu(R_i + C_j) + b2.
Boundary corrections:
    column j=0   uses R0   = W1_a x_i + W1_c x_{i-1} + b1     (drop e-term)
    column j=383 uses R383 = W1_a x_i + W1_e x_{i+1} + b1     (drop c-term)
    row i=0      uses CA   = W1_b x_j + W1_f x_{j-1}          (drop d-term)
    row i=383    uses CB   = W1_b x_j + W1_d x_{j+1}          (drop f-term)
    corners (0,0)/(383,383) need both (handled by patching R0/R383 columns).

Sharding: 8 cores, 48 output rows (i) each, both batches; every core gets the
full (transposed, zero-padded) x so the +-1 shifts are just AP column offsets.
The program is SPMD-uniform; core specialization (row-0/row-383 corrections)
enters only through per-core input data (S_CA3/S_CB3 zeroed on edge cores,
per-core x column slices).

Per (b,i) row the hidden tile relu(R_i + C) [128h x 384j] is produced by one
fused op (add + max 0) on DVE / ACT / GPSIMD (weighted round-robin), then
reduced against W2 by a PE matmul whose stationary is a [128,32] slice of a
zero slab with w2 at one column -> each row's result lands on its own PSUM
partition of a single [96, 384] accumulator bank (b2 pre-accumulated by a
contract-1 matmul).  One DVE copy extracts everything to SBUF.
"""

import ml_dtypes
import numpy as np
from contextlib import ExitStack

import concourse.bass as bass
import concourse.bacc as bacc
import concourse.tile as tile
from concourse import mybir
from concourse.bass_utils import run_bass_kernel_spmd

B, N, CIN, H = 2, 384, 64, 128
NCORES = 8
RPC = N // NCORES  # rows (i) per core = 48
ROWS = B * RPC     # (b, i) rows per core = 96

F32 = mybir.dt.float32
F32R = mybir.dt.float32r
BF16 = mybir.dt.bfloat16

# dtype of the C-side tensors and the relu'd hidden tiles.
CDT = BF16
MDT = BF16

ADD = mybir.AluOpType.add
MAX = mybir.AluOpType.max
RELU = mybir.ActivationFunctionType.Relu
IDENT = mybir.ActivationFunctionType.Identity

# main-loop row split across elementwise engines
N_DVE, N_ACT, N_POOL = 70, 26, 0
assert N_DVE + N_ACT + N_POOL == ROWS

USE_TILE_POSITION = True


def _engine_sequence():
    """Bresenham-interleave the 96 rows across the three engines."""
    quota = {e: n for e, n in
             (("v", N_DVE), ("s", N_ACT), ("g", N_POOL)) if n > 0}
    err = {e: 0.0 for e in quota}
    seq = []
    for _ in range(ROWS):
        for e in quota:
            err[e] += quota[e]
        best = max(err, key=lambda e: err[e])
        seq.append(best)
        err[best] -= ROWS
    return seq


def build_program(b2_value: float):
    """Build the SPMD Bass program (same NEFF for all 8 cores)."""
    nc = bacc.Bacc(
        "TRN2", target_bir_lowering=False, debug=False,
        enable_asserts=False, num_devices=NCORES,
    )
    # ---- DRAM I/O ----
    # constants are bundled by partition count so each lands in one DMA
    W64 = B * (N + 2) + B * (RPC + 2) + 8 * H   # xp | xsl | S_a..S_f,SCA3,SCB3
    W128 = 63 + 1                               # w2slab | w2t
    d_b64 = nc.dram_tensor("b64", [CIN, W64], F32R, kind="ExternalInput").ap()
    d_b128 = nc.dram_tensor("b128", [H, W128], BF16, kind="ExternalInput").ap()
    d_b1 = nc.dram_tensor("b1c", [H, 2], F32, kind="ExternalInput").ap()
    d_y = nc.dram_tensor("y", [B, RPC, N], F32, kind="ExternalOutput").ap()

    eng_seq = _engine_sequence()

    with tile.TileContext(nc) as tc, ExitStack() as ctx:
        consts = ctx.enter_context(tc.tile_pool(name="consts", bufs=1))
        cpool = ctx.enter_context(tc.tile_pool(name="cmats", bufs=1))
        rpool = ctx.enter_context(tc.tile_pool(name="rmats", bufs=1))
        mpool = ctx.enter_context(tc.tile_pool(name="mtiles", bufs=9))
        ypool = ctx.enter_context(tc.tile_pool(name="yout", bufs=1))
        ps = ctx.enter_context(tc.tile_pool(name="ps", bufs=6, space="PSUM"))
        yps_pool = ctx.enter_context(tc.tile_pool(name="yps", bufs=2, space="PSUM"))

        # ---- load constants ----
        W64 = B * (N + 2) + B * (RPC + 2) + 8 * H
        b64 = consts.tile([CIN, W64], F32R, tag="b64", name="b64")
        nc.sync.dma_start(b64[:, :], d_b64)
        b128 = consts.tile([H, 63 + 1], BF16, tag="b128", name="b128")
        nc.sync.dma_start(b128[:, :], d_b128)
        b1 = consts.tile([H, 2], F32, tag="b1c", name="b1c")
        nc.sync.dma_start(b1[:, :], d_b1)
        b2col = b1[:, 1:2]

        off = 0
        xp, xsl = [], []
        for b in range(B):
            xp.append(b64[:, off:off + N + 2])
            off += N + 2
        for b in range(B):
            xsl.append(b64[:, off:off + RPC + 2])
            off += RPC + 2
        S = {}
        for k in "abcdef":
            S[k] = b64[:, off:off + H]
            off += H
        SCA3 = b64[:, off:off + H]
        off += H
        SCB3 = b64[:, off:off + H]
        off += H
        assert off == W64
        w2slab = b128[:, 0:63]
        w2 = b128[:, 63:64]

        def mmr(out_ap, lhsT_ap, rhs_ap, start, stop, tile_position=None):
            nc.tensor.matmul(out_ap, lhsT_ap, rhs_ap,
                             start=start, stop=stop, tile_position=tile_position)

        # ---- per-batch setup: C, CA, CB, R, R0, R383 ----
        C_sb, CA_sb, CB_sb = [], [], []
        R_sb, R0_sb, R383_sb = [], [], []
        c0_f32, c383_f32 = [], []  # fp32 C columns 0/383 for the column fixes
        for b in range(B):
            xpb, xslb = xp[b], xsl[b]
            # C = S_b·x + S_d·x(+1) + S_f·x(-1)   [128, 384] in PSUM
            C_ps = ps.tile([H, N], F32, tag="ps", name="ps")
            mmr(C_ps[:, :], S["b"], xpb[:, 1:N + 1], True, False)
            mmr(C_ps[:, :], S["d"], xpb[:, 2:N + 2], False, False)
            mmr(C_ps[:, :], S["f"], xpb[:, 0:N], False, True)
            # CA = S_b·x + S_f·x(-1) + S_CA3·x(+1)  (S_CA3 = S_d, or 0 on core 0)
            CA_ps = ps.tile([H, N], F32, tag="ps", name="ps")
            mmr(CA_ps[:, :], S["b"], xpb[:, 1:N + 1], True, False)
            mmr(CA_ps[:, :], S["f"], xpb[:, 0:N], False, False)
            mmr(CA_ps[:, :], SCA3, xpb[:, 2:N + 2], False, True)
            # CB = S_b·x + S_d·x(+1) + S_CB3·x(-1)  (S_CB3 = S_f, or 0 on core 7)
            CB_ps = ps.tile([H, N], F32, tag="ps", name="ps")
            mmr(CB_ps[:, :], S["b"], xpb[:, 1:N + 1], True, False)
            mmr(CB_ps[:, :], S["d"], xpb[:, 2:N + 2], False, False)
            mmr(CB_ps[:, :], SCB3, xpb[:, 0:N], False, True)

            # R-side, on this core's 48-row slice
            R_ps = ps.tile([H, RPC], F32, tag="ps", name="ps")
            mmr(R_ps[:, :], S["a"], xslb[:, 1:RPC + 1], True, False)
            mmr(R_ps[:, :], S["c"], xslb[:, 0:RPC], False, True)
            r0 = rpool.tile([H, RPC], F32, tag=f"R0_{b}", name=f"R0_{b}")
            nc.scalar.activation(r0[:, :], R_ps[:, :], IDENT, bias=b1[:, 0:1])
            # continue accumulating the e-term onto the same PSUM -> full R
            mmr(R_ps[:, :], S["e"], xslb[:, 2:RPC + 2], False, True)
            r = rpool.tile([H, RPC], F32, tag=f"R_{b}", name=f"R_{b}")
            nc.scalar.activation(r[:, :], R_ps[:, :], IDENT, bias=b1[:, 0:1])
            R383_ps = ps.tile([H, RPC], F32, tag="ps", name="ps")
            mmr(R383_ps[:, :], S["a"], xslb[:, 1:RPC + 1], True, False)
            mmr(R383_ps[:, :], S["e"], xslb[:, 2:RPC + 2], False, True)
            r383 = rpool.tile([H, RPC], F32, tag=f"R383_{b}", name=f"R383_{b}")
            nc.scalar.activation(r383[:, :], R383_ps[:, :], IDENT, bias=b1[:, 0:1])
            R_sb.append(r)
            R0_sb.append(r0)
            R383_sb.append(r383)

            # copies PSUM -> SBUF (convert to CDT)
            c = cpool.tile([H, N], CDT, tag=f"C_{b}", name=f"C_{b}")
            nc.vector.tensor_copy(c[:, :], C_ps[:, :])
            ca = cpool.tile([H, N], CDT, tag=f"CA_{b}", name=f"CA_{b}")
            nc.vector.tensor_copy(ca[:, :], CA_ps[:, :])
            cb = cpool.tile([H, N], CDT, tag=f"CB_{b}", name=f"CB_{b}")
            nc.vector.tensor_copy(cb[:, :], CB_ps[:, :])
            C_sb.append(c)
            CA_sb.append(ca)
            CB_sb.append(cb)
            cc0 = rpool.tile([H, 1], F32, tag=f"c0_{b}", name=f"c0_{b}")
            nc.vector.tensor_copy(cc0[:, :], C_ps[:, 0:1])
            cc383 = rpool.tile([H, 1], F32, tag=f"c383_{b}", name=f"c383_{b}")
            nc.vector.tensor_copy(cc383[:, :], C_ps[:, N - 1:N])
            c0_f32.append(cc0)
            c383_f32.append(cc383)

            # corner patches: R0[:,0] += CA[:,0]-C[:,0]; R383[:,-1] += CB[:,-1]-C[:,-1]
            dca = rpool.tile([H, 1], F32, tag=f"dca_{b}", name=f"dca_{b}")
            nc.vector.tensor_sub(dca[:, :], ca[:, 0:1], c[:, 0:1])
            nc.vector.tensor_add(r0[:, 0:1], r0[:, 0:1], dca[:, :])
            dcb = rpool.tile([H, 1], F32, tag=f"dcb_{b}", name=f"dcb_{b}")
            nc.vector.tensor_sub(dcb[:, :], cb[:, N - 1:N], c[:, N - 1:N])
            nc.vector.tensor_add(r383[:, RPC - 1:RPC], r383[:, RPC - 1:RPC],
                                 dcb[:, :])

        # ---- main loop ----
        # All 96 row-results stack on distinct partitions of one PSUM bank.
        yacc = yps_pool.tile([ROWS, N], F32, tag="yacc", name="yacc")

        # visit rows rotating across the three 32-partition groups so the PE
        # array overlaps streams (different col_grp => different sub-arrays)
        visit = [(k % 3) * 32 + k // 3 for k in range(ROWS)]
        for k in range(ROWS):
            p = visit[k]
            b, i = divmod(p, RPC)
            if i == 0:
                cin = CA_sb[b]
            elif i == RPC - 1:
                cin = CB_sb[b]
            else:
                cin = C_sb[b]
            eng = eng_seq[k]
            m = mpool.tile([H, N], MDT, tag="m", name="m")
            rcol = R_sb[b][:, i:i + 1]
            if eng == "v":
                nc.vector.tensor_scalar(m[:, :], cin[:, :], rcol, 0.0, ADD, MAX)
            elif eng == "g":
                nc.gpsimd.tensor_scalar(m[:, :], cin[:, :], rcol, 0.0, ADD, MAX)
            else:
                nc.scalar.activation(m[:, :], cin[:, :], RELU, bias=rcol)
            g, c = divmod(p, 32)
            # stationary [128, 32] with w2 at column c: sliding slab slice
            stat = w2slab[:, 31 - c: 63 - c]
            tp = (0, 32 * g) if USE_TILE_POSITION else None
            nc.tensor.matmul(yacc[32 * g:32 * g + 32, :], stat, m[:, :],
                             start=k < 3, stop=k >= ROWS - 3, tile_position=tp)

        # extract all 96 rows at once, folding in +b2 (ACT: close to PSUM)
        Y = ypool.tile([ROWS, N], F32, tag="Y", name="Y")
        nc.scalar.activation(Y[:, :], yacc[:, :], IDENT, bias=b2col[0:ROWS, :])

        # ---- boundary columns j=0 and j=383 (both batches in one matmul) ----
        for col in (0, N - 1):
            mc = mpool.tile([H, ROWS], MDT, tag="mcol", name="mcol")
            for b in range(B):
                rt = R0_sb[b] if col == 0 else R383_sb[b]
                csc = c0_f32[b] if col == 0 else c383_f32[b]
                nc.vector.tensor_scalar(mc[:, b * RPC:(b + 1) * RPC], rt[:, :],
                                        csc[:, :], 0.0, ADD, MAX)
            yc_ps = ps.tile([ROWS, 1], F32, tag="ps", name="ps")
            nc.tensor.matmul(yc_ps[:, :], mc[:, :], w2, start=True, stop=True)
            nc.scalar.activation(Y[:, col:col + 1], yc_ps[:, :], IDENT,
                                 bias=b2col[0:ROWS, :])

        # ---- store ----
        nc.sync.dma_start(d_y.flatten_outer_dims(), Y[:, :])

    nc.compile()
    return nc


def _prep_inputs(x, W1, b1, W2, b2):
    """Host-side restructuring (layout only, no FLOPs)."""
    x = np.asarray(x, np.float32)
    W1 = np.asarray(W1, np.float32)
    b1 = np.asarray(b1, np.float32)
    W2 = np.asarray(W2, np.float32)
    b2v = float(np.asarray(b2).reshape(-1)[0])
    xp = np.zeros((B, CIN, N + 2), np.float32)
    xp[:, :, 1:N + 1] = x.transpose(0, 2, 1)
    S = {k: np.ascontiguousarray(W1[:, 64 * i:64 * (i + 1)].T)
         for i, k in enumerate("abcdef")}
    w2slab = np.zeros((H, 63), np.float32)
    w2slab[:, 31] = W2.reshape(H)
    b128 = np.concatenate([w2slab, W2.reshape(1, H).T], axis=1)
    b128 = b128.astype(ml_dtypes.bfloat16)
    b1c = np.concatenate([b1.reshape(H, 1),
                          np.full((H, 1), b2v, np.float32)], axis=1)
    return xp, S, b128, b1c, b2v


def kernel(x, W1, b1, W2, b2, trace=False):
    xp, S, b128, b1c, b2v = _prep_inputs(x, W1, b1, W2, b2)
    nc = build_program(b2v)

    zeros_s = np.zeros((CIN, H), np.float32)
    in_maps = []
    for c in range(NCORES):
        lo = c * RPC
        xsl = [xp[b, :, lo:lo + RPC + 2] for b in range(B)]
        sca3 = zeros_s if c == 0 else S["d"]
        scb3 = zeros_s if c == NCORES - 1 else S["f"]
        b64 = np.concatenate(
            [xp[0], xp[1]] + xsl + [S[k] for k in "abcdef"] + [sca3, scb3],
            axis=1)
        in_maps.append({
            "b64": np.ascontiguousarray(b64),
            "b128": b128,
            "b1c": b1c,
        })

    res = run_bass_kernel_spmd(nc, in_maps, core_ids=list(range(NCORES)),
                               trace=trace)
    y = np.concatenate([res.results[c]["y"] for c in range(NCORES)], axis=1)
    y = y.reshape(B, N, N, 1).astype(np.float32)
    if trace:
        return y, res
    return y


# revision 17
# speedup vs baseline: 3.3075x; 1.0558x over previous
"""Trainium2 Bass kernel for nn_FCPairedLayer (gnn_message_passing).

Reference computation:
    v[b,i,j] = concat(x_i, x_j, x_{i-1}*m1, x_{j+1}*m1, x_{i+1}*m2, x_{j-1}*m2)
    y[b,i,j] = W2 @ relu(W1 @ v + b1) + b2        (scalar output per pair)
with m1 = [i>=1][j<=N-2], m2 = [i<=N-2][j>=1].

W1 @ v splits into row-only and column-only terms; per batch define
    R[:,i] = W1_a x_i + W1_c x_{i-1} + W1_e x_{i+1} + b1     (shifts masked)
    C[:,j] = W1_b x_j + W1_d x_{j+1} + W1_f x_{j-1}
so that for interior cells  y[i,j] = W2 @ relu(R_i + C_j) + b2.
Boundary corrections:
    column j=0   uses R0   = W1_a x_i + W1_c x_{i-1} + b1     (drop e-term)
    column j=383 uses R383 = W1_a x_i + W1_e x_{i+1} + b1     (drop c-term)
    row i=0      uses CA   = W1_b x_j + W1_f x_{j-1}          (drop d-term)
    row i=383    uses CB   = W1_b x_j + W1_d x_{j+1}          (drop f-term)
    corners (0,0)/(383,383) need both (handled by patching R0/R383 columns).

Sharding: 8 cores, 48 output rows (i) each, both batches; every core gets the
full (transposed, zero-padded) x so the +-1 shifts are just AP column offsets.
The program is SPMD-uniform; core specialization (the row-0/row-383
corrections) enters only through per-core input data: difference stationaries
(S_d - S_CA3 etc.) are zero on interior cores, so there CA == C == CB.

C/CA/CB are built by ONE 5-matmul PSUM accumulation chain per batch (fp32r),
with SBUF snapshots (bf16 casts) taken at the CA / C / CB stages; R/R0/R383
likewise by one 4-matmul chain snapshotted via ACT bias-adds (+b1).

Per (b,i) row the hidden tile relu(R_i + C) [128h x 384j] is produced by one
fused op (add + max 0) on DVE or ACT (weighted round-robin), then reduced
against W2 by a PE matmul whose stationary is a [128,32] slice of a zero slab
with w2 at one column -> each row's result lands on its own partition of a
single [96, 384] PSUM accumulator (visits rotate the three 32-partition
col-groups so LDWEIGHTS overlaps the streaming matmuls).  One ACT pass
extracts PSUM -> SBUF folding in +b2.
"""

import ml_dtypes
import numpy as np
from contextlib import ExitStack

import concourse.bass as bass
import concourse.bacc as bacc
import concourse.tile as tile
from concourse import mybir
from concourse.bass_utils import run_bass_kernel_spmd

B, N, CIN, H = 2, 384, 64, 128
NCORES = 8
RPC = N // NCORES  # rows (i) per core = 48
ROWS = B * RPC     # (b, i) rows per core = 96

F32 = mybir.dt.float32
F32R = mybir.dt.float32r
BF16 = mybir.dt.bfloat16

ADD = mybir.AluOpType.add
MAX = mybir.AluOpType.max
RELU = mybir.ActivationFunctionType.Relu
IDENT = mybir.ActivationFunctionType.Identity

# main-loop row split across the two usable elementwise engines
N_DVE, N_ACT = 66, 30
assert N_DVE + N_ACT == ROWS

# S-matrix block order inside the 64-partition bundle
SBLOCKS = ["a", "b", "c", "e", "f", "CA3", "dd", "cbdiff", "negc"]


def _engine_sequence():
    """Bresenham-interleave the 96 rows across the engines."""
    quota = {"v": N_DVE, "s": N_ACT}
    err = {e: 0.0 for e in quota}
    seq = []
    for _ in range(ROWS):
        for e in quota:
            err[e] += quota[e]
        best = max(err, key=lambda e: err[e])
        seq.append(best)
        err[best] -= ROWS
    return seq


def build_program(b2_value: float):
    """Build the SPMD Bass program (same NEFF for all 8 cores)."""
    nc = bacc.Bacc(
        "TRN2", target_bir_lowering=False, debug=False,
        enable_asserts=False, num_devices=NCORES,
    )
    # ---- DRAM I/O (constants bundled by partition count: one DMA each) ----
    W64 = B * (N + 2) + B * (RPC + 2) + len(SBLOCKS) * H
    d_b64 = nc.dram_tensor("b64", [CIN, W64], F32R, kind="ExternalInput").ap()
    d_b128 = nc.dram_tensor("b128", [H, 64], BF16, kind="ExternalInput").ap()
    d_b1 = nc.dram_tensor("b1c", [H, 2], F32, kind="ExternalInput").ap()
    d_y = nc.dram_tensor("y", [B, RPC, N], F32, kind="ExternalOutput").ap()

    eng_seq = _engine_sequence()

    with tile.TileContext(nc) as tc, ExitStack() as ctx:
        consts = ctx.enter_context(tc.tile_pool(name="consts", bufs=1))
        cpool = ctx.enter_context(tc.tile_pool(name="cmats", bufs=1))
        rpool = ctx.enter_context(tc.tile_pool(name="rmats", bufs=1))
        mpool = ctx.enter_context(tc.tile_pool(name="mtiles", bufs=9))
        ypool = ctx.enter_context(tc.tile_pool(name="yout", bufs=1))
        ps = ctx.enter_context(tc.tile_pool(name="ps", bufs=5, space="PSUM"))
        yps_pool = ctx.enter_context(
            tc.tile_pool(name="yps", bufs=2, space="PSUM"))

        # ---- load constants ----
        b64 = consts.tile([CIN, W64], F32R, tag="b64", name="b64")
        nc.sync.dma_start(b64[:, :], d_b64)
        b128 = consts.tile([H, 64], BF16, tag="b128", name="b128")
        nc.sync.dma_start(b128[:, :], d_b128)
        b1 = consts.tile([H, 2], F32, tag="b1c", name="b1c")
        nc.sync.dma_start(b1[:, :], d_b1)
        b1col = b1[:, 0:1]
        b2col = b1[:, 1:2]

        off = 0
        xp, xsl = [], []
        for b in range(B):
            xp.append(b64[:, off:off + N + 2])
            off += N + 2
        for b in range(B):
            xsl.append(b64[:, off:off + RPC + 2])
            off += RPC + 2
        S = {}
        for k in SBLOCKS:
            S[k] = b64[:, off:off + H]
            off += H
        assert off == W64
        w2slab = b128[:, 0:63]
        w2 = b128[:, 63:64]

        def mmr(out_ap, lhsT_ap, rhs_ap, start, stop):
            nc.tensor.matmul(out_ap, lhsT_ap, rhs_ap, start=start, stop=stop)

        # ---- per-batch setup chains ----
        C_sb, CA_sb, CB_sb = [], [], []
        R_sb, R0_sb, R383_sb = [], [], []
        c0_f32, c383_f32 = [], []
        for b in range(B):
            xpb, xslb = xp[b], xsl[b]
            xm, xu, xd = xpb[:, 1:N + 1], xpb[:, 0:N], xpb[:, 2:N + 2]
            # wide chain: T -> +P_f -> CA -> C -> CB   [128, 384] PSUM
            W_ps = ps.tile([H, N], F32, tag="ps", name="ps")
            mmr(W_ps[:, :], S["b"], xm, True, False)      # P_b
            mmr(W_ps[:, :], S["f"], xu, False, False)     # + P_f
            mmr(W_ps[:, :], S["CA3"], xd, False, False)   # + S_CA3*x(+1) = CA
            ca = cpool.tile([H, N], BF16, tag=f"CA_{b}", name=f"CA_{b}")
            nc.vector.tensor_copy(ca[:, :], W_ps[:, :])
            mmr(W_ps[:, :], S["dd"], xd, False, False)    # + (S_d-S_CA3) = C
            c = cpool.tile([H, N], BF16, tag=f"C_{b}", name=f"C_{b}")
            nc.vector.tensor_copy(c[:, :], W_ps[:, :])
            mmr(W_ps[:, :], S["cbdiff"], xu, False, True)  # + (S_CB3-S_f) = CB
            cb = cpool.tile([H, N], BF16, tag=f"CB_{b}", name=f"CB_{b}")
            nc.vector.tensor_copy(cb[:, :], W_ps[:, :])
            C_sb.append(c)
            CA_sb.append(ca)
            CB_sb.append(cb)

            # fp32 column scalars for the column fixes (bf16-consistent)
            cc0 = rpool.tile([H, 1], F32, tag=f"c0_{b}", name=f"c0_{b}")
            nc.vector.tensor_copy(cc0[:, :], c[:, 0:1])
            cc383 = rpool.tile([H, 1], F32, tag=f"c383_{b}", name=f"c383_{b}")
            nc.vector.tensor_copy(cc383[:, :], c[:, N - 1:N])
            c0_f32.append(cc0)
            c383_f32.append(cc383)

            # small chain: R0 -> R -> R383   [128, 48] PSUM, +b1 via ACT copy
            xms, xus, xds = (xslb[:, 1:RPC + 1], xslb[:, 0:RPC],
                             xslb[:, 2:RPC + 2])
            R_ps = ps.tile([H, RPC], F32, tag="ps", name="ps")
            mmr(R_ps[:, :], S["a"], xms, True, False)     # P_a
            mmr(R_ps[:, :], S["c"], xus, False, False)    # + sh(P_c) = R0
            r0 = rpool.tile([H, RPC], F32, tag=f"R0_{b}", name=f"R0_{b}")
            nc.scalar.activation(r0[:, :], R_ps[:, :], IDENT, bias=b1col)
            mmr(R_ps[:, :], S["e"], xds, False, False)    # + sh(P_e) = R
            r = rpool.tile([H, RPC], F32, tag=f"R_{b}", name=f"R_{b}")
            nc.scalar.activation(r[:, :], R_ps[:, :], IDENT, bias=b1col)
            mmr(R_ps[:, :], S["negc"], xus, False, True)  # - sh(P_c) = R383
            r383 = rpool.tile([H, RPC], F32, tag=f"R383_{b}", name=f"R383_{b}")
            nc.scalar.activation(r383[:, :], R_ps[:, :], IDENT, bias=b1col)
            R_sb.append(r)
            R0_sb.append(r0)
            R383_sb.append(r383)

            # corner patches: R0[:,0] += CA[:,0]-C[:,0];
            #                 R383[:,-1] += CB[:,-1]-C[:,-1]
            dca = rpool.tile([H, 1], F32, tag=f"dca_{b}", name=f"dca_{b}")
            nc.vector.tensor_sub(dca[:, :], ca[:, 0:1], c[:, 0:1])
            nc.vector.tensor_add(r0[:, 0:1], r0[:, 0:1], dca[:, :])
            dcb = rpool.tile([H, 1], F32, tag=f"dcb_{b}", name=f"dcb_{b}")
            nc.vector.tensor_sub(dcb[:, :], cb[:, N - 1:N], c[:, N - 1:N])
            nc.vector.tensor_add(r383[:, RPC - 1:RPC], r383[:, RPC - 1:RPC],
                                 dcb[:, :])

        # ---- output staging ----
        Y = ypool.tile([ROWS, N], F32, tag="Y", name="Y")

        # ---- boundary columns j=0 / j=383 (before the strip loop) ----
        for col in (0, N - 1):
            mc = mpool.tile([H, ROWS], BF16, tag="mcol", name="mcol")
            for b in range(B):
                rt = R0_sb[b] if col == 0 else R383_sb[b]
                csc = c0_f32[b] if col == 0 else c383_f32[b]
                nc.vector.tensor_scalar(mc[:, b * RPC:(b + 1) * RPC], rt[:, :],
                                        csc[:, :], 0.0, ADD, MAX)
            yc_ps = ps.tile([ROWS, 1], F32, tag="ps", name="ps")
            nc.tensor.matmul(yc_ps[:, :], mc[:, :], w2, start=True, stop=True)
            nc.scalar.activation(Y[:, col:col + 1], yc_ps[:, :], IDENT,
                                 bias=b2col[0:ROWS, :])

        # ---- main loop: 96 rows, rotating the three 32-partition groups ----
        yacc = yps_pool.tile([ROWS, N], F32, tag="yacc", name="yacc")
        for k in range(ROWS):
            p = (k % 3) * 32 + k // 3
            b, i = divmod(p, RPC)
            if i == 0:
                cin = CA_sb[b]
            elif i == RPC - 1:
                cin = CB_sb[b]
            else:
                cin = C_sb[b]
            m = mpool.tile([H, N], BF16, tag="m", name="m")
            rcol = R_sb[b][:, i:i + 1]
            if eng_seq[k] == "v":
                nc.vector.tensor_scalar(m[:, :], cin[:, :], rcol, 0.0, ADD, MAX)
            else:
                nc.scalar.activation(m[:, :], cin[:, :], RELU, bias=rcol)
            g, col = divmod(p, 32)
            stat = w2slab[:, 31 - col: 63 - col]
            nc.tensor.matmul(yacc[32 * g:32 * g + 32, :], stat, m[:, :],
                             start=k < 3, stop=k >= ROWS - 3,
                             tile_position=(0, 32 * g))

        # extract rows (interior columns only), folding in +b2 on ACT
        nc.scalar.activation(Y[:, 1:N - 1], yacc[:, 1:N - 1], IDENT,
                             bias=b2col[0:ROWS, :])

        # ---- store ----
        nc.sync.dma_start(d_y.flatten_outer_dims(), Y[:, :])

    nc.compile()
    return nc


def _prep_inputs(x, W1, b1, W2, b2):
    """Host-side restructuring (layout only, no FLOPs beyond tiny S diffs)."""
    x = np.asarray(x, np.float32)
    W1 = np.asarray(W1, np.float32)
    b1 = np.asarray(b1, np.float32)
    W2 = np.asarray(W2, np.float32)
    b2v = float(np.asarray(b2).reshape(-1)[0])
    xp = np.zeros((B, CIN, N + 2), np.float32)
    xp[:, :, 1:N + 1] = x.transpose(0, 2, 1)
    S = {k: np.ascontiguousarray(W1[:, 64 * i:64 * (i + 1)].T)
         for i, k in enumerate("abcdef")}
    w2slab = np.zeros((H, 63), np.float32)
    w2slab[:, 31] = W2.reshape(H)
    b128 = np.concatenate([w2slab, W2.reshape(1, H).T], axis=1)
    b128 = b128.astype(ml_dtypes.bfloat16)
    b1c = np.concatenate([b1.reshape(H, 1),
                          np.full((H, 1), b2v, np.float32)], axis=1)
    return xp, S, b128, b1c, b2v


def kernel(x, W1, b1, W2, b2, trace=False):
    xp, S, b128, b1c, b2v = _prep_inputs(x, W1, b1, W2, b2)
    nc = build_program(b2v)

    zeros_s = np.zeros((CIN, H), np.float32)
    in_maps = []
    for c in range(NCORES):
        lo = c * RPC
        xsl = [xp[b, :, lo:lo + RPC + 2] for b in range(B)]
        sca3 = zeros_s if c == 0 else S["d"]
        scb3 = zeros_s if c == NCORES - 1 else S["f"]
        blocks = {
            "a": S["a"], "b": S["b"], "c": S["c"], "e": S["e"], "f": S["f"],
            "CA3": sca3,
            "dd": S["d"] - sca3,
            "cbdiff": scb3 - S["f"],
            "negc": -S["c"],
        }
        b64 = np.concatenate(
            [xp[0], xp[1]] + xsl + [blocks[k] for k in SBLOCKS], axis=1)
        in_maps.append({
            "b64": np.ascontiguousarray(b64),
            "b128": b128,
            "b1c": b1c,
        })

    res = run_bass_kernel_spmd(nc, in_maps, core_ids=list(range(NCORES)),
                               trace=trace)
    y = np.concatenate([res.results[c]["y"] for c in range(NCORES)], axis=1)
    y = y.reshape(B, N, N, 1).astype(np.float32)
    if trace:
        return y, res
    return y
